# revision 36
# baseline (speedup 1.0000x reference)
"""Trainium2 Bass kernel for nn_ABC_2D: hash-gather + per-pixel batched GEMM.

  out[b, k, p] = sum_c W[p, k, c] * x.flat[hashtable[b*P + p, c]]

Strategy (8 NeuronCores, SPMD):
  - Shard the pixel dimension: 512 pixels per core.
  - Host regroups the hash-gathered image values per pixel (index-driven
    data layout) and pre-transposes weights; all 9.7 GFLOP of the batched
    GEMM run on device.
  - Contraction 288 = 128 + 128 + 32: two full-width K=128 chunks plus a
    32-row tail. Tail data is DMA'd into a 32-row band of a [128, .]
    SBUF tile whose other 96 rows are zero (memset once per slot), so the
    tail matmul is also a plain K=128 matmul — one uniform PE mode, and
    every DMA except the 4 MB of tails spans all 128 partitions (317
    GB/s/core vs 235 at 96 partitions). The band rotates across the four
    partition quadrants per tile to balance SBUF write ports.
  - Even/odd pixels map to PE column tiles T0/T1 (tile_position (0,0) /
    (0,64)) so one tile's LDWEIGHTS overlaps the other's MATMUL, and the
    PSUM tile spans all 128 partitions for full-width DVE evacuation.
  - bf16 operands and bf16 output (fp32 PSUM accumulate);
    rel err vs the f32 reference ~2.9e-3.
"""
import sys

for _p in ("/opt/trn_rl_repo", "/root/.axon_site/_ro/trn_rl_repo"):
    if _p not in sys.path:
        sys.path.insert(0, _p)

import numpy as np
import ml_dtypes

import concourse.bass as bass
import concourse.tile as tile
from concourse import bacc, mybir
from concourse.bass_utils import run_bass_kernel_spmd

# Problem shape (hardcoded per spec)
B = 64          # batch
P = 4096        # pixel_number
KPP = 64        # kernels_per_pixel
CKS = 288       # C * kernel_size
NCORES = 8
PPC = P // NCORES          # 512 pixels per core
KC = 128                   # main contraction chunk rows
KT = CKS - 2 * KC          # 32 tail rows
PX = 64                    # pixels per SBUF tile
NT = PPC // PX             # 8 pixel tiles per core
GRP = 16                   # pixels per PSUM bank tile (2 x 8 pairs)

BF16 = mybir.dt.bfloat16
F32 = mybir.dt.float32

_NC_CACHE = {}


def _build_nc():
    if "nc" in _NC_CACHE:
        return _NC_CACHE["nc"]
    nc = bacc.Bacc(None, target_bir_lowering=False)

    g_par = [
        nc.declare_dram_parameter(f"g{j}", [KC, PPC * B], BF16, isOutput=False)
        for j in range(2)
    ]
    w_par = [
        nc.declare_dram_parameter(f"w{j}", [KC, PPC * KPP], BF16, isOutput=False)
        for j in range(2)
    ]
    # exact tails, thin layout [32, P*d]
    g2_par = nc.declare_dram_parameter(
        "g2", [KT, PPC * B], BF16, isOutput=False
    )
    w2_par = nc.declare_dram_parameter(
        "w2", [KT, PPC * KPP], BF16, isOutput=False
    )
    out_par = nc.declare_dram_parameter(
        "out", [2 * KPP, (PPC // 2) * B], BF16, isOutput=True
    )

    with tile.TileContext(nc) as tc:
        with (
            tc.tile_pool(name="gio", bufs=4) as gio,
            tc.tile_pool(name="wio", bufs=4) as wio,
            tc.tile_pool(name="oio", bufs=3) as oio,
            tc.tile_pool(name="ext", bufs=1) as ext,
            tc.tile_pool(name="ps", bufs=8, space="PSUM") as ps_pool,
        ):
            for t in range(NT):
                cols = slice(t * PX * B, (t + 1) * PX * B)
                ocols = slice(t * (PX // 2) * B, (t + 1) * (PX // 2) * B)
                g_t = []
                w_t = []
                for j in range(2):
                    gt = gio.tile([KC, PX * B], BF16, tag=f"g{j}")
                    nc.sync.dma_start(out=gt[:, :], in_=g_par[j][:, cols])
                    g_t.append(gt)
                    wt = wio.tile([KC, PX * KPP], BF16, tag=f"w{j}")
                    nc.sync.dma_start(out=wt[:, :], in_=w_par[j][:, cols])
                    w_t.append(wt)
                # padded tail tiles: rows 0-31 extracted bands, rows 32-127
                # zeroed so the K=128 tail matmul adds exactly the tail term
                band = t % 4   # rotate tail rows across partition quadrants
                eg = ext.tile([4 * KT, PX * B], BF16, tag=f"eg{band}")
                ew = ext.tile([4 * KT, PX * KPP], BF16, tag=f"ew{band}")
                if t < 4:      # zero rows persist per band slot
                    for qd in range(4):
                        if qd == band:
                            continue
                        qs = slice(qd * 32, (qd + 1) * 32)
                        nc.gpsimd.memset(eg[qs, :], 0.0)
                        nc.gpsimd.memset(ew[qs, :], 0.0)
                bs = slice(band * KT, (band + 1) * KT)
                nc.scalar.dma_start(out=eg[bs, :], in_=g2_par[:, cols])
                nc.scalar.dma_start(
                    out=ew[bs, :],
                    in_=w2_par[:, slice(t * PX * KPP, (t + 1) * PX * KPP)],
                )
                g_t.append(eg)
                w_t.append(ew)
                o_t = oio.tile([2 * KPP, (PX // 2) * B], BF16, tag="o")
                for grp in range(PX // GRP):
                    # [128, 512] PSUM tile: even pixel of each pair in
                    # partitions 0-63 (PE col-tile T0), odd in 64-127 (T1).
                    ps = ps_pool.tile([2 * KPP, (GRP // 2) * B],
                                      mybir.dt.float32, tag="ps")
                    for q in range(GRP):
                        lp = (grp * GRP + q) * B
                        half = q % 2
                        prow = slice(half * KPP, (half + 1) * KPP)
                        pcol = slice((q // 2) * B, (q // 2 + 1) * B)
                        for j in range(3):
                            nc.tensor.matmul(
                                ps[prow, pcol],
                                w_t[j][:, lp : lp + KPP],
                                g_t[j][:, lp : lp + B],
                                start=(j == 0),
                                stop=(j == 2),
                                tile_position=(0, half * KPP),
                            )
                    # o_t rows: even pixel k in partitions 0-63, odd in
                    # 64-127; col = pair_idx * B + b (unscrambled on host).
                    ob = slice(grp * (GRP // 2) * B, (grp + 1) * (GRP // 2) * B)
                    nc.vector.tensor_copy(o_t[:, ob], ps[:, :])
                nc.gpsimd.dma_start(
                    out=out_par[:KPP, ocols], in_=o_t[:KPP, :]
                )
                nc.scalar.dma_start(
                    out=out_par[KPP:, ocols], in_=o_t[KPP:, :]
                )
    nc.compile()
    _NC_CACHE["nc"] = nc
    return nc


def _prepare_in_maps(x, hashtable, weights):
    x = np.ascontiguousarray(np.asarray(x), dtype=np.float32)
    hashtable = np.asarray(hashtable)
    weights = np.asarray(weights, dtype=np.float32)

    # Hash-indexed regrouping of image values per pixel (data layout only).
    gathered = x.reshape(-1)[hashtable[: P * B]]            # (B*P, CKS) f32
    g_bf = gathered.astype(ml_dtypes.bfloat16)
    g_cpb = g_bf.reshape(B, P, CKS).transpose(2, 1, 0)      # (CKS, P, B)

    w_bf = weights.astype(ml_dtypes.bfloat16)
    w_cpk = w_bf.transpose(2, 0, 1)                         # (CKS, P, KPP)

    def tail_pack(src, pix, d):
        a = src[2 * KC :, pix, :]                            # (KT, PPC, d)
        return np.ascontiguousarray(a).reshape(KT, PPC * d)

    in_maps = []
    for i in range(NCORES):
        pix = slice(i * PPC, (i + 1) * PPC)
        m = {}
        for j in range(2):
            cs = slice(j * KC, (j + 1) * KC)
            m[f"g{j}"] = np.ascontiguousarray(g_cpb[cs, pix, :]).reshape(
                KC, PPC * B
            )
            m[f"w{j}"] = np.ascontiguousarray(w_cpk[cs, pix, :]).reshape(
                KC, PPC * KPP
            )
        m["g2"] = tail_pack(g_cpb, pix, B)
        m["w2"] = tail_pack(w_cpk, pix, KPP)
        in_maps.append(m)
    return in_maps


def _assemble(results):
    out = np.empty((B, KPP, P), dtype=np.float32)
    for i in range(NCORES):
        o = np.asarray(results[i]["out"]).astype(np.float32)
        o = o.reshape(2, KPP, PPC // 2, B)                  # (half, k, p2, b)
        out[:, :, i * PPC : (i + 1) * PPC] = o.transpose(3, 1, 2, 0).reshape(
            B, KPP, PPC
        )
    return out


def run(x, hashtable, weights, trace=False):
    nc = _build_nc()
    in_maps = _prepare_in_maps(x, hashtable, weights)
    res = run_bass_kernel_spmd(
        nc, in_maps, core_ids=list(range(NCORES)), trace=trace
    )
    return _assemble(res.results), res


def kernel(x, hashtable, weights):
    out, _ = run(x, hashtable, weights, trace=False)
    return out


# revision 37
# speedup vs baseline: 1.0317x; 1.0317x over previous
"""Trainium2 Bass kernel for nn_ABC_2D: hash-gather + per-pixel batched GEMM.

  out[b, k, p] = sum_c W[p, k, c] * x.flat[hashtable[b*P + p, c]]

Strategy (8 NeuronCores, SPMD):
  - Shard the pixel dimension: 512 pixels per core.
  - Host regroups the hash-gathered image values per pixel (index-driven
    data layout) and pre-transposes weights; all 9.7 GFLOP of the batched
    GEMM run on device.
  - Contraction 288 = 128 + 128 + 32: two full-width K=128 chunks plus a
    32-row tail. Tail data is DMA'd into a 32-row band of a [128, .]
    SBUF tile whose other 96 rows are zero (memset once per slot), so the
    tail matmul is also a plain K=128 matmul — one uniform PE mode, and
    every DMA except the 4 MB of tails spans all 128 partitions (317
    GB/s/core vs 235 at 96 partitions). The band rotates across the four
    partition quadrants per tile to balance SBUF write ports.
  - Even/odd pixels map to PE column tiles T0/T1 (tile_position (0,0) /
    (0,64)) so one tile's LDWEIGHTS overlaps the other's MATMUL, and the
    PSUM tile spans all 128 partitions for full-width DVE evacuation.
  - bf16 operands and bf16 output (fp32 PSUM accumulate);
    rel err vs the f32 reference ~2.9e-3.
"""
import sys

for _p in ("/opt/trn_rl_repo", "/root/.axon_site/_ro/trn_rl_repo"):
    if _p not in sys.path:
        sys.path.insert(0, _p)

import numpy as np
import ml_dtypes

import concourse.bass as bass
import concourse.tile as tile
from concourse import bacc, mybir
from concourse.bass_utils import run_bass_kernel_spmd

# Problem shape (hardcoded per spec)
B = 64          # batch
P = 4096        # pixel_number
KPP = 64        # kernels_per_pixel
CKS = 288       # C * kernel_size
NCORES = 8
PPC = P // NCORES          # 512 pixels per core
KC = 128                   # main contraction chunk rows
KT = CKS - 2 * KC          # 32 tail rows
PX = 64                    # pixels per SBUF tile
NT = PPC // PX             # 8 pixel tiles per core
GRP = 16                   # pixels per PSUM bank tile (2 x 8 pairs)

BF16 = mybir.dt.bfloat16
F32 = mybir.dt.float32

_NC_CACHE = {}


def _build_nc():
    if "nc" in _NC_CACHE:
        return _NC_CACHE["nc"]
    nc = bacc.Bacc(None, target_bir_lowering=False)

    # both K=128 main chunks merged per tile: one 16KB-descriptor DMA each
    g_par = nc.declare_dram_parameter(
        "g", [KC, 2 * PPC * B], BF16, isOutput=False
    )
    w_par = nc.declare_dram_parameter(
        "w", [KC, 2 * PPC * KPP], BF16, isOutput=False
    )
    # exact tails, thin layout [32, P*d]
    g2_par = nc.declare_dram_parameter(
        "g2", [KT, PPC * B], BF16, isOutput=False
    )
    w2_par = nc.declare_dram_parameter(
        "w2", [KT, PPC * KPP], BF16, isOutput=False
    )
    out_par = nc.declare_dram_parameter(
        "out", [2 * KPP, (PPC // 2) * B], BF16, isOutput=True
    )

    with tile.TileContext(nc) as tc:
        with (
            tc.tile_pool(name="gio", bufs=4) as gio,
            tc.tile_pool(name="wio", bufs=4) as wio,
            tc.tile_pool(name="oio", bufs=3) as oio,
            tc.tile_pool(name="ext", bufs=1) as ext,
            tc.tile_pool(name="ps", bufs=8, space="PSUM") as ps_pool,
        ):
            for t in range(NT):
                cols = slice(t * PX * B, (t + 1) * PX * B)
                ocols = slice(t * (PX // 2) * B, (t + 1) * (PX // 2) * B)
                gm = gio.tile([KC, 2 * PX * B], BF16, tag="g")
                nc.sync.dma_start(
                    out=gm[:, :],
                    in_=g_par[:, t * 2 * PX * B : (t + 1) * 2 * PX * B],
                )
                wm = wio.tile([KC, 2 * PX * KPP], BF16, tag="w")
                nc.sync.dma_start(
                    out=wm[:, :],
                    in_=w_par[:, t * 2 * PX * KPP : (t + 1) * 2 * PX * KPP],
                )
                g_t = [gm[:, : PX * B], gm[:, PX * B :]]
                w_t = [wm[:, : PX * KPP], wm[:, PX * KPP :]]
                # padded tail tiles: rows 0-31 extracted bands, rows 32-127
                # zeroed so the K=128 tail matmul adds exactly the tail term
                band = t % 4   # rotate tail rows across partition quadrants
                eg = ext.tile([4 * KT, PX * B], BF16, tag=f"eg{band}")
                ew = ext.tile([4 * KT, PX * KPP], BF16, tag=f"ew{band}")
                if t < 4:      # zero rows persist per band slot
                    for qd in range(4):
                        if qd == band:
                            continue
                        qs = slice(qd * 32, (qd + 1) * 32)
                        nc.gpsimd.memset(eg[qs, :], 0.0)
                        nc.gpsimd.memset(ew[qs, :], 0.0)
                bs = slice(band * KT, (band + 1) * KT)
                nc.scalar.dma_start(out=eg[bs, :], in_=g2_par[:, cols])
                nc.scalar.dma_start(
                    out=ew[bs, :],
                    in_=w2_par[:, slice(t * PX * KPP, (t + 1) * PX * KPP)],
                )
                g_t.append(eg)
                w_t.append(ew)
                o_t = oio.tile([2 * KPP, (PX // 2) * B], BF16, tag="o")
                for grp in range(PX // GRP):
                    # [128, 512] PSUM tile: even pixel of each pair in
                    # partitions 0-63 (PE col-tile T0), odd in 64-127 (T1).
                    ps = ps_pool.tile([2 * KPP, (GRP // 2) * B],
                                      mybir.dt.float32, tag="ps")
                    for q in range(GRP):
                        lp = (grp * GRP + q) * B
                        half = q % 2
                        prow = slice(half * KPP, (half + 1) * KPP)
                        pcol = slice((q // 2) * B, (q // 2 + 1) * B)
                        for j in range(3):
                            nc.tensor.matmul(
                                ps[prow, pcol],
                                w_t[j][:, lp : lp + KPP],
                                g_t[j][:, lp : lp + B],
                                start=(j == 0),
                                stop=(j == 2),
                                tile_position=(0, half * KPP),
                            )
                    # o_t rows: even pixel k in partitions 0-63, odd in
                    # 64-127; col = pair_idx * B + b (unscrambled on host).
                    ob = slice(grp * (GRP // 2) * B, (grp + 1) * (GRP // 2) * B)
                    nc.vector.tensor_copy(o_t[:, ob], ps[:, :])
                nc.gpsimd.dma_start(out=out_par[:, ocols], in_=o_t[:, :])
    nc.compile()
    _NC_CACHE["nc"] = nc
    return nc


def _prepare_in_maps(x, hashtable, weights):
    x = np.ascontiguousarray(np.asarray(x), dtype=np.float32)
    hashtable = np.asarray(hashtable)
    weights = np.asarray(weights, dtype=np.float32)

    # Hash-indexed regrouping of image values per pixel (data layout only).
    gathered = x.reshape(-1)[hashtable[: P * B]]            # (B*P, CKS) f32
    g_bf = gathered.astype(ml_dtypes.bfloat16)
    g_cpb = g_bf.reshape(B, P, CKS).transpose(2, 1, 0)      # (CKS, P, B)

    w_bf = weights.astype(ml_dtypes.bfloat16)
    w_cpk = w_bf.transpose(2, 0, 1)                         # (CKS, P, KPP)

    def tail_pack(src, pix, d):
        a = src[2 * KC :, pix, :]                            # (KT, PPC, d)
        return np.ascontiguousarray(a).reshape(KT, PPC * d)

    def main_merge(src, pix, d):
        # (2*KC, PPC, d) -> [KC, NT*2*PX*d]: per pixel tile, chunk0 block
        # then chunk1 block
        a = src[: 2 * KC, pix, :]                            # (256, PPC, d)
        a = a.reshape(2, KC, NT, PX, d)                      # (j, c, t, p, d)
        a = a.transpose(1, 2, 0, 3, 4)                       # (c, t, j, p, d)
        return np.ascontiguousarray(a).reshape(KC, 2 * PPC * d)

    in_maps = []
    for i in range(NCORES):
        pix = slice(i * PPC, (i + 1) * PPC)
        m = {
            "g": main_merge(g_cpb, pix, B),
            "w": main_merge(w_cpk, pix, KPP),
            "g2": tail_pack(g_cpb, pix, B),
            "w2": tail_pack(w_cpk, pix, KPP),
        }
        in_maps.append(m)
    return in_maps


def _assemble(results):
    out = np.empty((B, KPP, P), dtype=np.float32)
    for i in range(NCORES):
        o = np.asarray(results[i]["out"]).astype(np.float32)
        o = o.reshape(2, KPP, PPC // 2, B)                  # (half, k, p2, b)
        out[:, :, i * PPC : (i + 1) * PPC] = o.transpose(3, 1, 2, 0).reshape(
            B, KPP, PPC
        )
    return out


def run(x, hashtable, weights, trace=False):
    nc = _build_nc()
    in_maps = _prepare_in_maps(x, hashtable, weights)
    res = run_bass_kernel_spmd(
        nc, in_maps, core_ids=list(range(NCORES)), trace=trace
    )
    return _assemble(res.results), res


def kernel(x, hashtable, weights):
    out, _ = run(x, hashtable, weights, trace=False)
    return out
